# revision 4
# baseline (speedup 1.0000x reference)
"""Trainium2 Bass kernel for the ConsistencyStopController problem.

Computes, from s_i (512,256) and small weight matrices:
  action_logps (512,10,4), stop_logps (512,512,10,2), start_logps (512,10)

Sharding: data-parallel over the first T axis (start states i) across 8
NeuronCores; each core holds the full t (small) and computes its 64-row slice
of stop_logps plus its slice of the small outputs.

Device-side math for the big output:
  d[i,j,b]  = -||t_j - alpha_{i,b}||^2 = 2<alpha,t> - ||t||^2 - ||alpha||^2
computed as ONE matmul per 128-pair tile by extending the contraction dim:
  rows 0..63 : alpha2^T (2*alpha, transposed)   x  t^T
  row  64    : ones                             x  -||t_j||^2
  row  65    : -||alpha_p||^2                   x  ones
so PSUM holds d directly.  Then DVE copies d out, ACT computes e=Exp(d) and
om=Ln(1-e) (fused via activation scale=-1, bias=1).  Host side only does
layout (transpose/concat) for sharding and gathering.
"""

import sys

for _p in ("/opt/trn_rl_repo", "/opt/pypackages"):
    if _p not in sys.path:
        sys.path.insert(0, _p)

from contextlib import ExitStack

import numpy as np

import concourse.bacc as bacc
import concourse.bass as bass
import concourse.tile as tile
from concourse import mybir
from concourse import bass_utils

F32 = mybir.dt.float32
AF = mybir.ActivationFunctionType
ALU = mybir.AluOpType

T_DIM, S_DIM, TAU, B_DIM, A_DIM = 512, 256, 64, 10, 4
N_CORES = 8
ROWS = T_DIM // N_CORES          # 64 start-states per core
PAIRS = ROWS * B_DIM             # 640 (i,b) pairs per core, b-major: p = b*ROWS + i
N_PTILES = PAIRS // 128          # 5 pair-tiles of 128

# packed input column layouts
IP_COLS = TAU + B_DIM * A_DIM + ROWS        # W_tau | W_micro | sT_slice  = 168
WP_COLS = TAU + (TAU + B_DIM) + B_DIM       # W_trans[:64] | W_trans.T | W_pol = 148
BP_COLS = TAU + TAU + B_DIM * A_DIM + B_DIM  # b_tau | b_trans | b_micro | b_pol = 178

_CACHE = {}


def _build_body(ctx, tc, dr):
    nc = tc.nc
    sT_d, ip_d, wp_d, bp_d, stop_d, act_d, pol_d = dr

    const = ctx.enter_context(tc.tile_pool(name="const", bufs=1))
    work = ctx.enter_context(tc.tile_pool(name="work", bufs=3))
    small = ctx.enter_context(tc.tile_pool(name="small", bufs=2))
    psum = ctx.enter_context(tc.tile_pool(name="psum", bufs=2, space="PSUM"))

    # --- constants -------------------------------------------------------
    ones_row = const.tile([1, T_DIM], F32)       # 1.0 row, sliced as needed
    nc.vector.memset(ones_row, 1.0)
    negone_col = const.tile([TAU, 1], F32)       # -1.0 col (for -||t||^2)
    nc.vector.memset(negone_col, -1.0)
    negq_col = const.tile([TAU, 1], F32)         # -0.25 col (na from alpha2^2)
    nc.vector.memset(negq_col, -0.25)

    # warm the exp/ln activation table set early (overlaps with input DMAs)
    dummy = const.tile([1, 8], F32)
    nc.scalar.activation(dummy, ones_row[:, 0:8], AF.Exp)
    nc.scalar.activation(dummy, ones_row[:, 0:8], AF.Ln)

    # --- input loads -----------------------------------------------------
    sT0 = const.tile([128, T_DIM], F32)
    sT1 = const.tile([128, T_DIM], F32)
    nc.sync.dma_start(out=sT0, in_=sT_d[0:128, :])
    nc.sync.dma_start(out=sT1, in_=sT_d[128:256, :])
    ip0 = const.tile([128, IP_COLS], F32)
    ip1 = const.tile([128, IP_COLS], F32)
    nc.sync.dma_start(out=ip0, in_=ip_d[0:128, :])
    nc.sync.dma_start(out=ip1, in_=ip_d[128:256, :])
    wp = const.tile([TAU, WP_COLS], F32)
    nc.sync.dma_start(out=wp, in_=wp_d)
    bp = const.tile([1, BP_COLS], F32)
    nc.sync.dma_start(out=bp, in_=bp_d)

    W_tau = (ip0[:, 0:TAU], ip1[:, 0:TAU])
    W_micro = (ip0[:, TAU:TAU + 40], ip1[:, TAU:TAU + 40])
    sTs = (ip0[:, 104:IP_COLS], ip1[:, 104:IP_COLS])
    W_trans64 = wp[:, 0:TAU]                 # [k, k']
    WbT = wp[:, 2 * TAU:2 * TAU + B_DIM]     # W_trans.T tail -> [k', b]
    W_pol = wp[:, WP_COLS - B_DIM:WP_COLS]   # [k, b]
    b_tau_row = bp[:, 0:TAU]
    b_trans_row = bp[:, TAU:2 * TAU]
    b_micro_row = bp[:, 128:168]
    b_pol_row = bp[:, 168:178]

    # --- extended contraction layout (compute engines can only write at
    # base partitions that are multiples of 32, so pad K from 66 to 97):
    #   rows 0..63 : 2*alpha^T        x  t^T
    #   row  64    : -||alpha_p||^2   x  1
    #   rows 65..95: 0                x  (anything)
    #   row  96    : 1                x  -||t_j||^2
    KEXT = 97

    # --- tT_ext [97, 512]
    tT_ext = const.tile([KEXT, T_DIM], F32)
    ps_t = psum.tile([TAU, T_DIM], F32, tag="ps_big")
    nc.tensor.matmul(ps_t, W_tau[0], sT0, start=True, stop=False)
    nc.tensor.matmul(ps_t, W_tau[1], sT1, start=False, stop=False)
    nc.tensor.matmul(ps_t, b_tau_row, ones_row, start=False, stop=True)
    nc.vector.tensor_copy(tT_ext[0:TAU, :], ps_t)
    sq_t = const.tile([TAU, T_DIM], F32)
    nc.vector.tensor_mul(sq_t, tT_ext[0:TAU, :], tT_ext[0:TAU, :])
    nc.vector.memset(tT_ext[TAU:96, :], 1.0)
    ps_nt = psum.tile([1, T_DIM], F32, tag="ps_row")
    nc.tensor.matmul(ps_nt, negone_col, sq_t, start=True, stop=True)
    nc.vector.tensor_copy(tT_ext[96:KEXT, :], ps_nt)

    # --- tTs [64 k', 64 i]: slice t transposed (for alpha and start_logps)
    tTs = const.tile([TAU, ROWS], F32)
    ps_ts = psum.tile([TAU, ROWS], F32, tag="ps_sm")
    nc.tensor.matmul(ps_ts, W_tau[0], sTs[0], start=True, stop=False)
    nc.tensor.matmul(ps_ts, W_tau[1], sTs[1], start=False, stop=False)
    nc.tensor.matmul(ps_ts, b_tau_row, ones_row[:, 0:ROWS], start=False, stop=True)
    nc.vector.tensor_copy(tTs, ps_ts)

    # --- alpha_ext [97, 640]: rows 0..63 = 2*alpha^T, row 64 = -na,
    # rows 65..95 = 0, row 96 = 1.  Pair index p = b*64 + i  (b-major).
    alpha_ext = const.tile([KEXT, PAIRS], F32)
    ps_ab = psum.tile([TAU, ROWS], F32, tag="ps_sm")
    nc.tensor.matmul(ps_ab, W_trans64, tTs, start=True, stop=False)
    nc.tensor.matmul(ps_ab, b_trans_row, ones_row[:, 0:ROWS], start=False, stop=True)
    abT2 = const.tile([TAU, ROWS], F32)
    nc.vector.tensor_scalar(out=abT2, in0=ps_ab, scalar1=2.0, scalar2=None,
                            op0=ALU.mult)
    Wb2 = const.tile([TAU, B_DIM], F32)
    nc.vector.tensor_scalar(out=Wb2, in0=WbT, scalar1=2.0, scalar2=None,
                            op0=ALU.mult)
    for b in range(B_DIM):
        nc.vector.tensor_scalar(
            out=alpha_ext[0:TAU, b * ROWS:(b + 1) * ROWS],
            in0=abT2, scalar1=Wb2[:, b:b + 1], scalar2=None, op0=ALU.add)
    nc.vector.memset(alpha_ext[TAU:96, :], 0.0)
    nc.vector.memset(alpha_ext[96:KEXT, :], 1.0)
    asq = const.tile([TAU, PAIRS], F32)
    nc.vector.tensor_mul(asq, alpha_ext[0:TAU, :], alpha_ext[0:TAU, :])
    ps_na0 = psum.tile([1, 512], F32, tag="ps_row")
    nc.tensor.matmul(ps_na0, negq_col, asq[:, 0:512], start=True, stop=True)
    nc.vector.tensor_copy(alpha_ext[TAU:TAU + 1, 0:512], ps_na0)
    ps_na1 = psum.tile([1, PAIRS - 512], F32, tag="ps_row")
    nc.tensor.matmul(ps_na1, negq_col, asq[:, 512:PAIRS], start=True, stop=True)
    nc.vector.tensor_copy(alpha_ext[TAU:TAU + 1, 512:PAIRS], ps_na1)

    # --- big loop: 5 pair-tiles of 128; psum gets d directly -------------
    for m in range(N_PTILES):
        ps_d = psum.tile([128, T_DIM], F32, tag="ps_d")
        nc.tensor.matmul(ps_d, alpha_ext[:, m * 128:(m + 1) * 128], tT_ext,
                         start=True, stop=True)
        out_sb = work.tile([128, 2 * T_DIM], F32, tag="out_sb")
        nc.vector.tensor_copy(out_sb[:, 0:T_DIM], ps_d)
        e_sb = work.tile([128, T_DIM], F32, tag="e_sb")
        nc.scalar.activation(e_sb, ps_d, AF.Exp)
        # om = Ln(1 - e)  (scale=-1, bias=1 fused into the activation)
        nc.scalar.activation(out_sb[:, T_DIM:2 * T_DIM], e_sb, AF.Ln,
                             bias=1.0, scale=-1.0)
        nc.sync.dma_start(out=stop_d[m * 128:(m + 1) * 128, :], in_=out_sb)

    # --- action_logps slice: micro net + grouped log-softmax -------------
    NBA = B_DIM * A_DIM
    ps_mic = psum.tile([ROWS, NBA], F32, tag="ps_sm")
    nc.tensor.matmul(ps_mic, sTs[0], W_micro[0], start=True, stop=False)
    nc.tensor.matmul(ps_mic, sTs[1], W_micro[1], start=False, stop=False)
    nc.tensor.matmul(ps_mic, ones_row[:, 0:ROWS], b_micro_row,
                     start=False, stop=True)
    mic = small.tile([ROWS, NBA], F32)
    nc.vector.tensor_copy(mic, ps_mic)
    mic3 = mic.rearrange("p (b a) -> p b a", a=A_DIM)
    mx = small.tile([ROWS, B_DIM], F32)
    nc.vector.reduce_max(mx, mic3, axis=mybir.AxisListType.X)
    mx_b = bass.AP(tensor=mx.tensor, offset=mx.offset,
                   ap=[mx.ap[0], mx.ap[1], [0, A_DIM]])
    shifted = small.tile([ROWS, NBA], F32)
    shifted3 = shifted.rearrange("p (b a) -> p b a", a=A_DIM)
    nc.vector.tensor_tensor(out=shifted3, in0=mic3, in1=mx_b, op=ALU.subtract)
    ex = small.tile([ROWS, NBA], F32)
    nc.scalar.activation(ex, shifted, AF.Exp)
    ssum = small.tile([ROWS, B_DIM], F32)
    nc.vector.reduce_sum(ssum, ex.rearrange("p (b a) -> p b a", a=A_DIM),
                         axis=mybir.AxisListType.X)
    lns = small.tile([ROWS, B_DIM], F32)
    nc.scalar.activation(lns, ssum, AF.Ln)
    lns_b = bass.AP(tensor=lns.tensor, offset=lns.offset,
                    ap=[lns.ap[0], lns.ap[1], [0, A_DIM]])
    act_o = small.tile([ROWS, NBA], F32)
    nc.vector.tensor_tensor(out=act_o.rearrange("p (b a) -> p b a", a=A_DIM),
                            in0=shifted3, in1=lns_b, op=ALU.subtract)
    nc.sync.dma_start(out=act_d, in_=act_o)

    # --- start_logps slice: policy net + log-softmax over b --------------
    ps_pol = psum.tile([ROWS, B_DIM], F32, tag="ps_sm")
    nc.tensor.matmul(ps_pol, tTs, W_pol, start=True, stop=False)
    nc.tensor.matmul(ps_pol, ones_row[:, 0:ROWS], b_pol_row,
                     start=False, stop=True)
    pol = small.tile([ROWS, B_DIM], F32)
    nc.vector.tensor_copy(pol, ps_pol)
    mx1 = small.tile([ROWS, 1], F32)
    nc.vector.reduce_max(mx1, pol, axis=mybir.AxisListType.X)
    sh1 = small.tile([ROWS, B_DIM], F32)
    nc.vector.tensor_scalar(out=sh1, in0=pol, scalar1=mx1, scalar2=None,
                            op0=ALU.subtract)
    ex1 = small.tile([ROWS, B_DIM], F32)
    nc.scalar.activation(ex1, sh1, AF.Exp)
    ss1 = small.tile([ROWS, 1], F32)
    nc.vector.reduce_sum(ss1, ex1, axis=mybir.AxisListType.X)
    ln1 = small.tile([ROWS, 1], F32)
    nc.scalar.activation(ln1, ss1, AF.Ln)
    pol_o = small.tile([ROWS, B_DIM], F32)
    nc.vector.tensor_scalar(out=pol_o, in0=sh1, scalar1=ln1, scalar2=None,
                            op0=ALU.subtract)
    nc.sync.dma_start(out=pol_d, in_=pol_o)


def build_bass():
    """Build and compile the (single, SPMD) Bass program."""
    if "nc" in _CACHE:
        return _CACHE["nc"]
    nc = bacc.Bacc("TRN2", target_bir_lowering=False, debug=False,
                   num_devices=N_CORES)
    sT_d = nc.dram_tensor("sT", (S_DIM, T_DIM), F32, kind="ExternalInput").ap()
    ip_d = nc.dram_tensor("inpack", (S_DIM, IP_COLS), F32,
                          kind="ExternalInput").ap()
    wp_d = nc.dram_tensor("wpack", (TAU, WP_COLS), F32,
                          kind="ExternalInput").ap()
    bp_d = nc.dram_tensor("bpack", (1, BP_COLS), F32, kind="ExternalInput").ap()
    stop_d = nc.dram_tensor("stop_out", (PAIRS, 2 * T_DIM), F32,
                            kind="ExternalOutput").ap()
    act_d = nc.dram_tensor("act_out", (ROWS, B_DIM * A_DIM), F32,
                           kind="ExternalOutput").ap()
    pol_d = nc.dram_tensor("pol_out", (ROWS, B_DIM), F32,
                           kind="ExternalOutput").ap()
    dr = (sT_d, ip_d, wp_d, bp_d, stop_d, act_d, pol_d)
    with tile.TileContext(nc) as tc:
        with ExitStack() as ctx:
            _build_body(ctx, tc, dr)
    nc.compile()
    _CACHE["nc"] = nc
    return nc


def make_in_maps(s_i, W_tau, b_tau, W_micro, b_micro, W_pol, b_pol,
                 W_trans, b_trans):
    """Host-side input sharding: pure layout (transpose / concat / slice)."""
    f32 = np.float32
    sT = np.ascontiguousarray(np.asarray(s_i, dtype=f32).T)          # (256,512)
    W_tau = np.asarray(W_tau, dtype=f32)
    W_micro = np.asarray(W_micro, dtype=f32)
    W_trans = np.asarray(W_trans, dtype=f32)
    W_pol = np.asarray(W_pol, dtype=f32)
    wpack = np.ascontiguousarray(
        np.concatenate([W_trans[:TAU], W_trans.T, W_pol], axis=1))   # (64,148)
    bpack = np.ascontiguousarray(
        np.concatenate([np.asarray(b_tau, dtype=f32),
                        np.asarray(b_trans, dtype=f32),
                        np.asarray(b_micro, dtype=f32),
                        np.asarray(b_pol, dtype=f32)])[None, :])     # (1,178)
    in_maps = []
    for c in range(N_CORES):
        sl = sT[:, c * ROWS:(c + 1) * ROWS]                          # (256,64)
        ipack = np.ascontiguousarray(
            np.concatenate([W_tau, W_micro, sl], axis=1))            # (256,168)
        in_maps.append({"sT": sT, "inpack": ipack, "wpack": wpack,
                        "bpack": bpack})
    return in_maps


def gather(results):
    """Host-side gather: pure layout (reshape / transpose / concat)."""
    f32 = np.float32
    action = np.empty((T_DIM, B_DIM, A_DIM), dtype=f32)
    stop = np.empty((T_DIM, T_DIM, B_DIM, 2), dtype=f32)
    start = np.empty((T_DIM, B_DIM), dtype=f32)
    for c, res in enumerate(results):
        rs = slice(c * ROWS, (c + 1) * ROWS)
        action[rs] = res["act_out"].reshape(ROWS, B_DIM, A_DIM)
        start[rs] = res["pol_out"]
        so = res["stop_out"]                       # (640, 1024), b-major pairs
        d = so[:, :T_DIM].reshape(B_DIM, ROWS, T_DIM)
        om = so[:, T_DIM:].reshape(B_DIM, ROWS, T_DIM)
        stop[rs, :, :, 0] = d.transpose(1, 2, 0)   # (i, j, b)
        stop[rs, :, :, 1] = om.transpose(1, 2, 0)
    return action, stop, start


def kernel(s_i, W_tau, b_tau, W_micro, b_micro, W_pol, b_pol, W_trans,
           b_trans, _trace=False):
    nc = build_bass()
    in_maps = make_in_maps(s_i, W_tau, b_tau, W_micro, b_micro, W_pol, b_pol,
                           W_trans, b_trans)
    res = bass_utils.run_bass_kernel_spmd(nc, in_maps,
                                          core_ids=list(range(N_CORES)),
                                          trace=_trace)
    _CACHE["last_results"] = res
    return gather(res.results)
